# revision 44
# baseline (speedup 1.0000x reference)
"""Trainium2 Bass kernel for nn_MemoryEfficientS6Compressor.

Key insight: the reference returns LN(W_out @ mean(ys[-8:]) + b_out) where
ys[-8:] are the last 8 positions of the LAST chunk (chunk-local t=24..31).
Chunks are independent, so only chunk 3 matters, and within it only:
  - xi (W_in proj) for chunk-local positions 14..31  (18 positions)
  - conv+silu (xc) for positions 17..31              (15 positions)
  - dt / gate / window-softmax for positions 24..31  (8 positions)

Sharding: 7 conv groups (351 channels) -> cores 0..6; core 7 runs the same
SPMD program on zeroed weights. Cross-core sums (x_proj partials, W_out
partials) via AllReduce.

Schedule (all fp16 matmuls; bf16 only for exp-range tiles):
  1. DMA the xp-critical slices first (x cols 448.., W_in block) so the
     x_proj AllReduce launches ~35us in instead of ~90us.
  2. A-passes over tokens 448..960, 960..1152 -> conv(448..960) -> xp ->
     AllReduce(fp16).  While it flies: A-pass 0..448, conv(0..448),
     dt-proj, gate.
  3. Windowed softmax on merged [128, 3*512] tiles (3 channel-chunks per
     op) split across Act (table exps) / DVE / Pool.
  4. out partial -> fp16 AllReduce (Shared output) -> layernorm.
"""

import os

import numpy as np

import concourse.bass as bass
import concourse.mybir as mybir
from concourse import bacc
import concourse.bass_utils as _BU
from concourse.bass_utils import run_bass_kernel_spmd

if os.environ.get("K_LDWOPT", "0") == "1" and not hasattr(_BU, "_k_ldw_patch"):
    _BU._k_ldw_patch = _BU.run_command

    def _run_command_ldw(argv, **kwargs):
        argv = ["--enable-ldw-opt=true" if a == "--enable-ldw-opt=false"
                else a for a in argv]
        return _BU._k_ldw_patch(argv, **kwargs)

    _BU.run_command = _run_command_ldw
from concourse.tile import TileContext

F32 = mybir.dt.float32
F16 = mybir.dt.float16
BF16 = mybir.dt.bfloat16
AF = mybir.ActivationFunctionType
ALU = mybir.AluOpType

SEQ, BATCH, D_MODEL = 128, 64, 2048
D_INNER, GROUPS, D_CONV = 2457, 7, 4
DT_RANK, WIN = 32, 8
GC = D_INNER // GROUPS          # 351 channels per group
NPOS = 18                        # xi positions (chunk-local 14..31)
NCONV = 15                       # conv output positions (17..31)
TOK = NPOS * BATCH               # 1152
TOKC = NCONV * BATCH             # 960
TOKZ = 8 * BATCH                 # 512
CH = [(0, 128), (128, 128), (256, 95)]
NK = D_MODEL // 128              # 16 k-chunks over d_model
HI = 448                         # token split: [448..1152] is xp-critical

_cache = {}


def _build():
    nc = bacc.Bacc("TRN2", target_bir_lowering=False, debug=False,
                   num_devices=8)

    xT = nc.dram_tensor("xT", [D_MODEL, TOK], F16, kind="ExternalInput").ap()
    wig = nc.dram_tensor("wig", [D_MODEL, 2 * GC], F16, kind="ExternalInput").ap()
    wc = nc.dram_tensor("wc", [GC, D_CONV * GC], F16, kind="ExternalInput").ap()
    wo = nc.dram_tensor("wo", [GC + 1, D_MODEL + 1], F16, kind="ExternalInput").ap()
    wx = nc.dram_tensor("wx", [GC, DT_RANK], F16, kind="ExternalInput").ap()
    wdt = nc.dram_tensor("wdt", [DT_RANK, GC], F16, kind="ExternalInput").ap()
    biasv = nc.dram_tensor("biasv", [GC, 4], F32, kind="ExternalInput").ap()
    bxp = nc.dram_tensor("bxp", [DT_RANK, 1], F32, kind="ExternalInput").ap()
    lnwb = nc.dram_tensor("lnwb", [2, D_MODEL], F16, kind="ExternalInput").ap()
    out = nc.dram_tensor("out", [BATCH, D_MODEL], F32, kind="ExternalOutput").ap()

    xp_part = nc.dram_tensor("xp_part", [DT_RANK, TOKZ], F16,
                             kind="Internal").ap()
    xp_red = nc.dram_tensor("xp_red", [DT_RANK, TOKZ], F16,
                            kind="Internal", addr_space="Shared").ap()
    op_part = nc.dram_tensor("op_part", [BATCH, D_MODEL + 1], F16,
                             kind="Internal").ap()
    op_red = nc.dram_tensor("op_red", [BATCH, D_MODEL + 1], F16,
                            kind="Internal", addr_space="Shared").ap()

    with TileContext(nc) as tc:
        with (
            tc.tile_pool(name="xt", bufs=1) as xt_pool,
            tc.tile_pool(name="wig", bufs=1) as wig_pool,
            tc.tile_pool(name="wgt", bufs=1) as wgt_pool,
            tc.tile_pool(name="wo", bufs=1) as wo_pool,
            tc.tile_pool(name="act", bufs=1) as act_pool,
            tc.tile_pool(name="ek", bufs=1) as ek_pool,
            tc.tile_pool(name="tmp", bufs=3) as tmp_pool,
            tc.tile_pool(name="sc", bufs=1) as sc_pool,
            tc.tile_pool(name="ps", bufs=1, space="PSUM") as ps_pool,
        ):
            # ---- DMA: xp-critical first (x hi-cols + W_in), paired k ------
            # host packs wig = [W_in.T | W_gate.T] (each [2048, 351])
            xthi_sb, wxi_sb = [], []
            for p in range(NK // 4):
                th = xt_pool.tile([128, 4, TOK - HI], F16, tag=f"xth{p}",
                                  name=f"xth{p}")
                nc.sync.dma_start(
                    out=th[:, :, :],
                    in_=xT[512 * p:512 * (p + 1), HI:].rearrange(
                        "(four p) t -> p four t", four=4))
                xthi_sb.append(th)
                tw = wig_pool.tile([128, 4, GC], F16, tag=f"wxi{p}",
                                   name=f"wxi{p}")
                nc.sync.dma_start(
                    out=tw[:, :, :],
                    in_=wig[512 * p:512 * (p + 1), 0:GC].rearrange(
                        "(four p) t -> p four t", four=4))
                wxi_sb.append(tw)
            # deferred: x lo-cols + W_gate
            xtlo_sb, wz_sb = [], []
            # (W_gate tiles loaded later, reusing the W_in slots)
            for p in range(NK // 4):
                tl = xt_pool.tile([128, 4, HI], F16, tag=f"xtl{p}",
                                  name=f"xtl{p}")
                nc.sync.dma_start(
                    out=tl[:, :, :],
                    in_=xT[512 * p:512 * (p + 1), 0:HI].rearrange(
                        "(four p) t -> p four t", four=4))
                xtlo_sb.append(tl)

            def xt_hi(k, t0, t1):       # tokens in [HI, 1152)
                return xthi_sb[k // 4][:, k % 4, t0 - HI:t1 - HI]

            def xt_lo(k, t0, t1):       # tokens in [0, HI)
                return xtlo_sb[k // 4][:, k % 4, t0:t1]

            def w_xi(k, c0, cw):
                return wxi_sb[k // 4][:, k % 4, c0:c0 + cw]

            def w_z(k, c0, cw):
                return wz_sb[k // 4][:, k % 4, c0:c0 + cw]

            # small tiles on the scalar queue
            bias_sb = []
            for m, (c0, cw) in enumerate(CH):
                b = sc_pool.tile([cw, 4], F32, tag=f"bias{m}", name=f"bias{m}")
                nc.scalar.dma_start(out=b[:], in_=biasv[c0:c0 + cw, :])
                bias_sb.append(b)
            wx_sb = []
            for m, (c0, cw) in enumerate(CH):
                t = sc_pool.tile([cw, DT_RANK], F16, tag=f"wx{m}", name=f"wx{m}")
                nc.scalar.dma_start(out=t[:], in_=wx[c0:c0 + cw, :])
                wx_sb.append(t)
            wdt_sb = sc_pool.tile([DT_RANK, GC], F16, tag="wdt", name="wdt")
            nc.scalar.dma_start(out=wdt_sb[:], in_=wdt[:, :])
            bxp_sb = sc_pool.tile([DT_RANK, 1], F32, tag="bxp", name="bxp")
            nc.scalar.dma_start(out=bxp_sb[:], in_=bxp[:, :])
            lnw_sb = sc_pool.tile([1, D_MODEL], F16, tag="lnw", name="lnw")
            nc.scalar.dma_start(out=lnw_sb[:], in_=lnwb[0:1, :])
            lnb_sb = sc_pool.tile([1, D_MODEL], F16, tag="lnb", name="lnb")
            nc.scalar.dma_start(out=lnb_sb[:], in_=lnwb[1:2, :])

            ones1 = sc_pool.tile([1, BATCH], F16, tag="ones1", name="ones1")
            nc.vector.memset(ones1[:], 1.0)
            cb = sc_pool.tile([128, 1], F32, tag="cb", name="cb")
            nc.vector.memset(cb[:, 0:1], 1e-5)

            # conv + out weights on the gpsimd queue
            wc_sb = []
            for m, (c0, cw) in enumerate(CH):
                t = wgt_pool.tile([cw, D_CONV * GC], F16, tag=f"wc{m}",
                                  name=f"wc{m}")
                nc.gpsimd.dma_start(out=t[:], in_=wc[c0:c0 + cw, :])
                wc_sb.append(t)


            # merged activation tiles: [128, 3*N] spanning channel-chunks
            xi_sb = [act_pool.tile([cw, TOK], F16, tag=f"xi{m}", name=f"xi{m}")
                     for m, (c0, cw) in enumerate(CH)]
            xc_all = act_pool.tile([128, 3 * TOKC], F16, tag="xc", name="xc")
            sigz_all = act_pool.tile([128, 3 * TOKZ], F16, tag="sigz",
                                     name="sigz")
            usp_all = act_pool.tile([128, 3 * TOKZ], F16, tag="usp", name="usp")
            nc.vector.memset(usp_all[64:128, 2 * TOKZ:3 * TOKZ], 0.0)
            nc.vector.memset(sigz_all[64:128, 2 * TOKZ:3 * TOKZ], 0.0)
            nc.vector.memset(xc_all[64:128, 2 * TOKC:3 * TOKC], 0.0)
            dt_all = act_pool.tile([128, 3 * TOKZ], F32, tag="dt", name="dt")
            EE = ek_pool.tile([128, 7 * 3 * TOKZ], BF16, tag="EE", name="EE")

            def xc3(m, t0, t1):
                return xc_all[0:CH[m][1], m * TOKC + t0:m * TOKC + t1]

            def xcv(k):     # [128, 3, 512] window view
                v = xc_all[:, :].rearrange("p (m t) -> p m t", m=3)
                return v[:, :, k * BATCH:k * BATCH + TOKZ]

            def eev(k):     # [128, 3, 512] view of plane k
                v = EE[:, (k - 1) * 3 * TOKZ:k * 3 * TOKZ]
                return v.rearrange("p (m t) -> p m t", m=3)

            def ee2(k):     # [128, 1536] view of plane k
                return EE[:, (k - 1) * 3 * TOKZ:k * 3 * TOKZ]

            # ---- A-pass over hi tokens (448..960 + 960..1152 share one
            # ---- ldweights per (k, m)), then conv(448..960), xp, AR --------
            pxi = [ps_pool.tile([cw, 512], F32, tag=f"pxi{m}",
                                name=f"pxi{m}_hi")
                   for m, (c0, cw) in enumerate(CH)]
            pxj = [ps_pool.tile([cw, 192], F32, tag="pc", bufs=3,
                                name=f"pxj{m}_hi")
                   for m, (c0, cw) in enumerate(CH)]
            for k in range(NK):
                for m, (c0, cw) in enumerate(CH):
                    nc.tensor.matmul(pxi[m][:], w_xi(k, c0, cw),
                                     xt_hi(k, 448, 960),
                                     start=(k == 0), stop=(k == NK - 1))
                    nc.tensor.matmul(pxj[m][:], w_xi(k, c0, cw),
                                     xt_hi(k, 960, 1152),
                                     start=(k == 0), stop=(k == NK - 1),
                                     skip_group_check=True)
            for m, (c0, cw) in enumerate(CH):
                nc.scalar.activation(xi_sb[m][:, 448:960], pxi[m][:],
                                     AF.Identity, bias=bias_sb[m][:, 0:1])
                nc.scalar.activation(xi_sb[m][:, 960:1152], pxj[m][:],
                                     AF.Identity, bias=bias_sb[m][:, 0:1])

            def a_pass(t0, t1, xt_f):
                pxa = [ps_pool.tile([cw, t1 - t0], F32, tag=f"pxi{m}",
                                    name=f"pxi{m}_{t0}")
                       for m, (c0, cw) in enumerate(CH)]
                for k in range(NK):
                    for m, (c0, cw) in enumerate(CH):
                        nc.tensor.matmul(pxa[m][:], w_xi(k, c0, cw),
                                         xt_f(k, t0, t1),
                                         start=(k == 0), stop=(k == NK - 1))
                for m, (c0, cw) in enumerate(CH):
                    nc.scalar.activation(xi_sb[m][:, t0:t1], pxa[m][:],
                                         AF.Identity, bias=bias_sb[m][:, 0:1])

            def conv(t0, tw):           # conv outputs for tokens [t0, t0+tw)
                for m, (c0, cw) in enumerate(CH):
                    pc = ps_pool.tile([cw, tw], F32, tag="pc", bufs=3,
                                      name=f"pconv{t0}_{m}")
                    for kc, (k0, kw) in enumerate(CH):
                        for j in range(D_CONV):
                            nc.tensor.matmul(
                                pc[:],
                                wc_sb[kc][:, j * GC + c0:j * GC + c0 + cw],
                                xi_sb[kc][:, t0 + j * BATCH:
                                           t0 + j * BATCH + tw],
                                start=(kc == 0 and j == 0),
                                stop=(kc == 2 and j == D_CONV - 1))
                    nc.scalar.activation(xc3(m, t0, t0 + tw), pc[:], AF.Silu,
                                         bias=bias_sb[m][:, 1:2])

            conv(448, 512)

            pxp = ps_pool.tile([DT_RANK, TOKZ], F32, tag="pc", bufs=3,
                               name="pxp")
            for kc, (k0, kw) in enumerate(CH):
                nc.tensor.matmul(pxp[:], wx_sb[kc][:], xc3(kc, 448, 960),
                                 start=(kc == 0), stop=(kc == 2))
            xp_sb = sc_pool.tile([DT_RANK, TOKZ], F16, tag="xp", name="xp")
            nc.scalar.activation(xp_sb[:], pxp[:], AF.Identity,
                                 bias=bxp_sb[:, 0:1])
            nc.sync.dma_start(out=xp_part[:], in_=xp_sb[:])
            nc.gpsimd.collective_compute(
                "AllReduce", ALU.add, replica_groups=[list(range(8))],
                ins=[xp_part.opt()], outs=[xp_red.opt()])
            xps = sc_pool.tile([DT_RANK, TOKZ], F16, tag="xps", name="xps")
            nc.gpsimd.dma_start(out=xps[:], in_=xp_red[:])
            wo_rows = [(0, 128), (128, 128), (256, 95), (351, 1)]
            wo_sb = []
            for r, (r0, rw) in enumerate(wo_rows):
                t = wo_pool.tile([rw, D_MODEL + 1], F16, tag=f"wo{r}",
                                 name=f"wo{r}")
                nc.gpsimd.dma_start(out=t[:], in_=wo[r0:r0 + rw, :])
                wo_sb.append(t)

            # ---- while the AR flies: rest of A, conv(0..448), dt-proj, gate
            a_pass(0, 448, xt_lo)
            # W_gate loads reuse the W_in tile slots: the WAR dependency on
            # the A-pass reads keeps these transfers out of the contended
            # startup window (they land just before the gate needs them).
            for p in range(NK // 4):
                tz = wig_pool.tile([128, 4, GC], F16, tag=f"wxi{p}",
                                   name=f"wz{p}")
                nc.sync.dma_start(
                    out=tz[:, :, :],
                    in_=wig[512 * p:512 * (p + 1), GC:].rearrange(
                        "(four p) t -> p four t", four=4))
                wz_sb.append(tz)
            conv(0, 448)

            for m, (c0, cw) in enumerate(CH):
                pz = ps_pool.tile([cw, TOKZ], F32, tag="pc", bufs=3, name="pz")
                for k in range(NK):
                    nc.tensor.matmul(pz[:], w_z(k, c0, cw),
                                     xt_hi(k, TOK - TOKZ, TOK),
                                     start=(k == 0), stop=(k == NK - 1))
                nc.scalar.activation(sigz_all[0:cw, m * TOKZ:(m + 1) * TOKZ],
                                     pz[:], AF.Sigmoid,
                                     bias=bias_sb[m][:, 2:3])

            for m, (c0, cw) in enumerate(CH):
                pdt = ps_pool.tile([cw, TOKZ], F32, tag="pc", bufs=3,
                                   name=f"pdt{m}")
                nc.tensor.matmul(pdt[:], wdt_sb[:, c0:c0 + cw], xps[:],
                                 start=True, stop=True)
                nc.scalar.activation(usp_all[0:cw, m * TOKZ:(m + 1) * TOKZ],
                                     pdt[:], AF.Exp)
            # E_k = (1+e^u)^k via one Identity + squares (all same act table)
            nc.scalar.activation(ee2(2), usp_all[:], AF.Square, bias=1.0)
            nc.scalar.activation(ee2(1), usp_all[:], AF.Identity, bias=1.0)
            nc.scalar.activation(ee2(4), ee2(2), AF.Square)

            # ---- windowed softmax on merged tiles --------------------------
            # S = 1 + sum_k E_k ; num = sum_k E_k * xc<<k ; E_k from Act.
            W3 = 3 * TOKZ
            S = act_pool.tile([128, W3], BF16, tag="S", name="S")
            num = act_pool.tile([128, W3], BF16, tag="num", name="num")
            r3 = lambda ap: ap.rearrange("p (m t) -> p m t", m=3)
            # odd powers + S = 1 + sum E_k, interleaved so S closes early
            nc.vector.tensor_scalar_add(S[:], usp_all[:], 2.0)
            nc.vector.tensor_mul(ee2(3), ee2(1), ee2(2))
            nc.vector.tensor_add(S[:], S[:], ee2(2))
            nc.vector.tensor_add(S[:], S[:], ee2(3))
            nc.vector.tensor_mul(ee2(5), ee2(1), ee2(4))
            nc.scalar.activation(ee2(6), ee2(3), AF.Square)
            nc.vector.tensor_add(S[:], S[:], ee2(4))
            nc.vector.tensor_add(S[:], S[:], ee2(5))
            nc.vector.tensor_mul(ee2(7), ee2(3), ee2(4))
            nc.vector.tensor_add(S[:], S[:], ee2(6))
            nc.vector.tensor_add(S[:], S[:], ee2(7))
            # pairwise product tree for num
            tvs = {}
            for k in range(1, 8):
                tv = tmp_pool.tile([128, W3], BF16, tag="tmp", bufs=8,
                                   name=f"t{k}")
                nc.vector.tensor_mul(r3(tv[:]), eev(k), xcv(k))
                tvs[k] = tv
            a10 = tvs[1]
            nc.vector.tensor_add(r3(a10[:]), r3(tvs[1][:]), xcv(0))
            a32 = tvs[3]
            nc.vector.tensor_add(a32[:], tvs[3][:], tvs[2][:])
            a54 = tvs[5]
            nc.vector.tensor_add(a54[:], tvs[5][:], tvs[4][:])
            a76 = tvs[7]
            nc.vector.tensor_add(a76[:], tvs[7][:], tvs[6][:])
            nc.vector.tensor_add(a10[:], a10[:], a32[:])
            nc.vector.tensor_add(a54[:], a54[:], a76[:])
            nc.vector.tensor_add(num[:], a10[:], a54[:])

            sf = tmp_pool.tile([128, W3], F32, tag="sf", bufs=1, name="sf")
            nc.scalar.copy(sf[:], S[:])
            sinv = tmp_pool.tile([128, W3], F32, tag="sinv", bufs=1,
                                 name="sinv")
            nc.vector.reciprocal_approx_fast(out=sinv[:], in_=sf[:])
            loc = tmp_pool.tile([128, W3], F16, tag="loc", bufs=1, name="loc")
            nc.vector.tensor_mul(loc[:], num[:], sinv[:])
            ys = tmp_pool.tile([128, W3], F16, tag="ys", bufs=1, name="ys")
            for m, (c0, cw) in enumerate(CH):
                nc.vector.scalar_tensor_tensor(
                    ys[0:cw, m * TOKZ:(m + 1) * TOKZ],
                    xc3(m, 448, 960), bias_sb[m][:, 3:4],
                    loc[0:cw, m * TOKZ:(m + 1) * TOKZ],
                    op0=ALU.mult, op1=ALU.add)
            nc.vector.tensor_mul(ys[:], ys[:], sigz_all[:])
            ysv = ys[:, :].rearrange("p (m two t) -> p m two t", m=3, two=2)
            tr1 = tmp_pool.tile([128, 3 * 256], F16, tag="tr1", bufs=1, name="tr1")
            nc.vector.tensor_add(
                tr1[:, :].rearrange("p (m t) -> p m t", m=3),
                ysv[:, :, 0], ysv[:, :, 1])
            t1v = tr1[:, :].rearrange("p (m two t) -> p m two t", m=3, two=2)
            tr2 = tmp_pool.tile([128, 3 * 128], F16, tag="tr2", bufs=1, name="tr2")
            nc.vector.tensor_add(
                tr2[:, :].rearrange("p (m t) -> p m t", m=3),
                t1v[:, :, 0], t1v[:, :, 1])
            t2v = tr2[:, :].rearrange("p (m two t) -> p m two t", m=3, two=2)
            cext_all = sc_pool.tile([128, 3 * BATCH], F16, tag="cext",
                                    name="cext")
            nc.vector.tensor_add(
                cext_all[:, :].rearrange("p (m t) -> p m t", m=3),
                t2v[:, :, 0], t2v[:, :, 1])



            # ---- out partial = cext @ woT (+b_out row), AllReduce ----------
            po = [ps_pool.tile([BATCH, 512], F32,
                               tag=(f"pxi{n}" if n < 3 else "po3"),
                               name=f"po{n}")
                  for n in range(4)]
            pomu = ps_pool.tile([BATCH, 1], F32, tag="pc", bufs=3,
                                name="pomu")
            for kc in range(4):
                lhs = (cext_all[0:CH[kc][1], kc * BATCH:(kc + 1) * BATCH]
                       if kc < 3 else ones1[:])
                for n in range(4):
                    nc.tensor.matmul(po[n][:], lhs,
                                     wo_sb[kc][:, n * 512:(n + 1) * 512],
                                     start=(kc == 0), stop=(kc == 3))
                nc.tensor.matmul(pomu[:], lhs,
                                 wo_sb[kc][:, D_MODEL:D_MODEL + 1],
                                 start=(kc == 0), stop=(kc == 3),
                                 skip_group_check=True)
            outp = sc_pool.tile([BATCH, D_MODEL + 1], F16, tag="outp",
                                name="outp")
            osb = sc_pool.tile([BATCH, D_MODEL + 1], F16, tag="osb",
                               name="osb")
            for n in range(2):
                nc.scalar.activation(outp[:, n * 512:(n + 1) * 512],
                                     po[n][:], AF.Copy)
            for n in range(2, 4):
                nc.vector.tensor_copy(outp[:, n * 512:(n + 1) * 512],
                                      po[n][:])
            nc.scalar.activation(outp[:, D_MODEL:D_MODEL + 1], pomu[:],
                                 AF.Copy)
            nc.sync.dma_start(out=op_part[:], in_=outp[:])
            nc.gpsimd.collective_compute(
                "AllReduce", ALU.add, replica_groups=[list(range(8))],
                ins=[op_part.opt()], outs=[op_red.opt()])
            nc.gpsimd.dma_start(out=osb[:], in_=op_red[:])

            # ---- layernorm over d_model (free dim) -------------------------
            mus = sc_pool.tile([BATCH, 1], F32, tag="mus", name="mus")
            nc.scalar.mul(mus[:], osb[:, D_MODEL:D_MODEL + 1], 1.0 / D_MODEL)
            nmus = sc_pool.tile([BATCH, 1], F32, tag="nmus", name="nmus")
            nc.scalar.mul(nmus[:], osb[:, D_MODEL:D_MODEL + 1], -1.0 / D_MODEL)
            cen = sc_pool.tile([BATCH, D_MODEL], F16, tag="cen", name="cen")
            nc.vector.tensor_scalar_sub(cen[:], osb[:, 0:D_MODEL], mus[:])
            sq = sc_pool.tile([BATCH, D_MODEL], F16, tag="outp", name="sq")
            vs = sc_pool.tile([BATCH, 1], F32, tag="vs", name="vs")
            nc.scalar.activation(sq[:], osb[:, 0:D_MODEL], AF.Square,
                                 bias=nmus[:, 0:1], accum_out=vs[:])
            std = sc_pool.tile([BATCH, 1], F32, tag="std", name="std")
            nc.scalar.activation(std[:], vs[:], AF.Sqrt,
                                 scale=1.0 / D_MODEL, bias=cb[0:BATCH, 0:1])
            rstd = sc_pool.tile([BATCH, 1], F32, tag="rstd", name="rstd")
            nc.vector.reciprocal(rstd[:], std[:])
            for n in range(4):
                pw = ps_pool.tile([BATCH, 512], F32,
                                  tag=(f"pxi{n}" if n < 3 else "po3"),
                                  name="pw")
                pb = ps_pool.tile([BATCH, 512], F32, tag="pc", bufs=3,
                                  name="pb")
                nc.tensor.matmul(pw[:], ones1[:],
                                 lnw_sb[:, n * 512:(n + 1) * 512],
                                 start=True, stop=True)
                nc.tensor.matmul(pb[:], ones1[:],
                                 lnb_sb[:, n * 512:(n + 1) * 512],
                                 start=True, stop=True)
                fin = sc_pool.tile([BATCH, 512], F32, tag="fin", bufs=4,
                                   name=f"fin{n}")
                eng = nc.vector
                eng.scalar_tensor_tensor(
                    fin[:], cen[:, n * 512:(n + 1) * 512], rstd[:], pw[:],
                    op0=ALU.mult, op1=ALU.mult)
                eng.tensor_add(fin[:], fin[:], pb[:])
                nc.sync.dma_start(out=out[:, n * 512:(n + 1) * 512],
                                  in_=fin[:])

    nc.compile()
    return nc


def _host_prep(inputs):
    f = lambda k: np.ascontiguousarray(np.asarray(inputs[k], dtype=np.float32))
    x, W_in, b_in = f("x"), f("W_in"), f("b_in")
    W_gate, b_gate = f("W_gate"), f("b_gate")
    W_conv, b_conv = f("W_conv"), f("b_conv")
    W_xproj, b_xproj = f("W_xproj"), f("b_xproj")
    W_dt, Dparam = f("W_dt"), f("Dparam")
    W_out, b_out = f("W_out"), f("b_out")
    ln_w, ln_b = f("ln_w"), f("ln_b")

    xT = np.ascontiguousarray(
        x[SEQ - NPOS:].reshape(TOK, D_MODEL).T).astype(np.float16)
    lnwb = np.ascontiguousarray(np.stack([ln_w, ln_b])).astype(np.float16)

    in_maps = []
    for g in range(8):
        if g < GROUPS:
            ch = slice(GC * g, GC * (g + 1))
            wigm = np.concatenate([W_in[ch].T, W_gate[ch].T], axis=1)
            wcm = np.ascontiguousarray(
                W_conv[ch].transpose(1, 2, 0).reshape(GC, D_CONV * GC))
            wom = np.zeros((GC + 1, D_MODEL + 1), np.float32)
            wom[:GC, :D_MODEL] = W_out[:, ch].T / float(WIN)
            if g == 0:
                wom[GC, :D_MODEL] = b_out
            wom[:, D_MODEL] = wom[:, :D_MODEL].sum(axis=1)
            wxm = np.ascontiguousarray(W_xproj[:DT_RANK, ch].T)
            wdtm = np.ascontiguousarray(W_dt[ch].T)
            biasm = np.ascontiguousarray(
                np.stack([b_in[ch], b_conv[ch], b_gate[ch], Dparam[ch]], 1))
            bxpm = (b_xproj[:DT_RANK] if g == 0
                    else np.zeros(DT_RANK, np.float32)).reshape(DT_RANK, 1)
            bxpm = np.ascontiguousarray(bxpm)
        else:
            wigm = np.zeros((D_MODEL, 2 * GC), np.float32)
            wcm = np.zeros((GC, D_CONV * GC), np.float32)
            wom = np.zeros((GC + 1, D_MODEL + 1), np.float32)
            wxm = np.zeros((GC, DT_RANK), np.float32)
            wdtm = np.zeros((DT_RANK, GC), np.float32)
            biasm = np.zeros((GC, 4), np.float32)
            bxpm = np.zeros((DT_RANK, 1), np.float32)
        in_maps.append({
            "xT": xT,
            "wig": np.ascontiguousarray(wigm).astype(np.float16),
            "wc": wcm.astype(np.float16),
            "wo": wom.astype(np.float16),
            "wx": wxm.astype(np.float16),
            "wdt": wdtm.astype(np.float16),
            "biasv": biasm, "bxp": bxpm, "lnwb": lnwb,
        })
    return in_maps


def kernel(**inputs):
    if "nc" not in _cache:
        _cache["nc"] = _build()
    in_maps = _host_prep(inputs)
    res = run_bass_kernel_spmd(_cache["nc"], in_maps, list(range(8)))
    return res.results[0]["out"]


# revision 45
# speedup vs baseline: 1.1028x; 1.1028x over previous
"""Trainium2 Bass kernel for nn_MemoryEfficientS6Compressor.

Key insight: the reference returns LN(W_out @ mean(ys[-8:]) + b_out) where
ys[-8:] are the last 8 positions of the LAST chunk (chunk-local t=24..31).
Chunks are independent, so only chunk 3 matters, and within it only:
  - xi (W_in proj) for chunk-local positions 14..31  (18 positions)
  - conv+silu (xc) for positions 17..31              (15 positions)
  - dt / gate / window-softmax for positions 24..31  (8 positions)

Sharding: 7 conv groups (351 channels) -> cores 0..6; core 7 runs the same
SPMD program on zeroed weights. Cross-core sums (x_proj partials, W_out
partials) via AllReduce.

Schedule (all fp16 matmuls; bf16 only for exp-range tiles):
  1. DMA the xp-critical slices first (x cols 448.., W_in block) so the
     x_proj AllReduce launches ~35us in instead of ~90us.
  2. A-passes over tokens 448..960, 960..1152 -> conv(448..960) -> xp ->
     AllReduce(fp16).  While it flies: A-pass 0..448, conv(0..448),
     dt-proj, gate.
  3. Windowed softmax on merged [128, 3*512] tiles (3 channel-chunks per
     op) split across Act (table exps) / DVE / Pool.
  4. out partial -> fp16 AllReduce (Shared output) -> layernorm.
"""

import os

import numpy as np

import concourse.bass as bass
import concourse.mybir as mybir
from concourse import bacc
import concourse.bass_utils as _BU
from concourse.bass_utils import run_bass_kernel_spmd

if os.environ.get("K_LDWOPT", "0") == "1" and not hasattr(_BU, "_k_ldw_patch"):
    _BU._k_ldw_patch = _BU.run_command

    def _run_command_ldw(argv, **kwargs):
        argv = ["--enable-ldw-opt=true" if a == "--enable-ldw-opt=false"
                else a for a in argv]
        return _BU._k_ldw_patch(argv, **kwargs)

    _BU.run_command = _run_command_ldw
from concourse.tile import TileContext

F32 = mybir.dt.float32
F16 = mybir.dt.float16
BF16 = mybir.dt.bfloat16
AF = mybir.ActivationFunctionType
ALU = mybir.AluOpType

SEQ, BATCH, D_MODEL = 128, 64, 2048
D_INNER, GROUPS, D_CONV = 2457, 7, 4
DT_RANK, WIN = 32, 8
GC = D_INNER // GROUPS          # 351 channels per group
NPOS = 18                        # xi positions (chunk-local 14..31)
NCONV = 15                       # conv output positions (17..31)
TOK = NPOS * BATCH               # 1152
TOKC = NCONV * BATCH             # 960
TOKZ = 8 * BATCH                 # 512
CH = [(0, 128), (128, 128), (256, 95)]
NK = D_MODEL // 128              # 16 k-chunks over d_model
HI = 448                         # token split: [448..1152] is xp-critical

_cache = {}


def _build():
    nc = bacc.Bacc("TRN2", target_bir_lowering=False, debug=False,
                   num_devices=8)

    xT = nc.dram_tensor("xT", [D_MODEL, TOK], F16, kind="ExternalInput").ap()
    wig = nc.dram_tensor("wig", [D_MODEL, 2 * GC], F16, kind="ExternalInput").ap()
    wc = nc.dram_tensor("wc", [GC, D_CONV * GC], F16, kind="ExternalInput").ap()
    wo = nc.dram_tensor("wo", [GC + 1, D_MODEL + 1], F16, kind="ExternalInput").ap()
    wx = nc.dram_tensor("wx", [GC, DT_RANK], F16, kind="ExternalInput").ap()
    wdt = nc.dram_tensor("wdt", [DT_RANK, GC], F16, kind="ExternalInput").ap()
    biasv = nc.dram_tensor("biasv", [GC, 4], F32, kind="ExternalInput").ap()
    bxp = nc.dram_tensor("bxp", [DT_RANK, 1], F32, kind="ExternalInput").ap()
    lnwb = nc.dram_tensor("lnwb", [2, D_MODEL], F16, kind="ExternalInput").ap()
    out = nc.dram_tensor("out", [BATCH, D_MODEL], F32, kind="ExternalOutput").ap()

    xp_part = nc.dram_tensor("xp_part", [DT_RANK, TOKZ], F16,
                             kind="Internal").ap()
    xp_red = nc.dram_tensor("xp_red", [DT_RANK, TOKZ], F16,
                            kind="Internal", addr_space="Shared").ap()
    op_part = nc.dram_tensor("op_part", [BATCH, D_MODEL + 1], F16,
                             kind="Internal").ap()
    op_red = nc.dram_tensor("op_red", [BATCH, D_MODEL + 1], F16,
                            kind="Internal", addr_space="Shared").ap()

    with TileContext(nc) as tc:
        with (
            tc.tile_pool(name="xt", bufs=1) as xt_pool,
            tc.tile_pool(name="wig", bufs=1) as wig_pool,
            tc.tile_pool(name="wgt", bufs=1) as wgt_pool,
            tc.tile_pool(name="wo", bufs=1) as wo_pool,
            tc.tile_pool(name="act", bufs=1) as act_pool,
            tc.tile_pool(name="ek", bufs=1) as ek_pool,
            tc.tile_pool(name="tmp", bufs=3) as tmp_pool,
            tc.tile_pool(name="sc", bufs=1) as sc_pool,
            tc.tile_pool(name="ps", bufs=1, space="PSUM") as ps_pool,
        ):
            # ---- DMA: xp-critical first (x hi-cols + W_in), paired k ------
            # host packs wig = [W_in.T | W_gate.T] (each [2048, 351])
            xthi_sb, wxi_sb = [], []
            for p in range(NK // 4):
                th = xt_pool.tile([128, 4, TOK - HI], F16, tag=f"xth{p}",
                                  name=f"xth{p}")
                nc.sync.dma_start(
                    out=th[:, :, :],
                    in_=xT[512 * p:512 * (p + 1), HI:].rearrange(
                        "(four p) t -> p four t", four=4))
                xthi_sb.append(th)
                tw = wig_pool.tile([128, 4, GC], F16, tag=f"wxi{p}",
                                   name=f"wxi{p}")
                nc.sync.dma_start(
                    out=tw[:, :, :],
                    in_=wig[512 * p:512 * (p + 1), 0:GC].rearrange(
                        "(four p) t -> p four t", four=4))
                wxi_sb.append(tw)
            # deferred: x lo-cols + W_gate
            xtlo_sb, wz_sb = [], []
            # (W_gate tiles loaded later, reusing the W_in slots)
            for p in range(NK // 4):
                tl = xt_pool.tile([128, 4, HI], F16, tag=f"xtl{p}",
                                  name=f"xtl{p}")
                nc.sync.dma_start(
                    out=tl[:, :, :],
                    in_=xT[512 * p:512 * (p + 1), 0:HI].rearrange(
                        "(four p) t -> p four t", four=4))
                xtlo_sb.append(tl)

            def xt_hi(k, t0, t1):       # tokens in [HI, 1152)
                return xthi_sb[k // 4][:, k % 4, t0 - HI:t1 - HI]

            def xt_lo(k, t0, t1):       # tokens in [0, HI)
                return xtlo_sb[k // 4][:, k % 4, t0:t1]

            def w_xi(k, c0, cw):
                return wxi_sb[k // 4][:, k % 4, c0:c0 + cw]

            def w_z(k, c0, cw):
                return wz_sb[k // 4][:, k % 4, c0:c0 + cw]

            # small tiles on the scalar queue
            bias_sb = []
            for m, (c0, cw) in enumerate(CH):
                b = sc_pool.tile([cw, 4], F32, tag=f"bias{m}", name=f"bias{m}")
                nc.scalar.dma_start(out=b[:], in_=biasv[c0:c0 + cw, :])
                bias_sb.append(b)
            wx_sb = []
            for m, (c0, cw) in enumerate(CH):
                t = sc_pool.tile([cw, DT_RANK], F16, tag=f"wx{m}", name=f"wx{m}")
                nc.scalar.dma_start(out=t[:], in_=wx[c0:c0 + cw, :])
                wx_sb.append(t)
            wdt_sb = sc_pool.tile([DT_RANK, GC], F16, tag="wdt", name="wdt")
            nc.scalar.dma_start(out=wdt_sb[:], in_=wdt[:, :])
            bxp_sb = sc_pool.tile([DT_RANK, 1], F32, tag="bxp", name="bxp")
            nc.scalar.dma_start(out=bxp_sb[:], in_=bxp[:, :])
            lnw_sb = sc_pool.tile([1, D_MODEL], F16, tag="lnw", name="lnw")
            nc.scalar.dma_start(out=lnw_sb[:], in_=lnwb[0:1, :])
            lnb_sb = sc_pool.tile([1, D_MODEL], F16, tag="lnb", name="lnb")
            nc.scalar.dma_start(out=lnb_sb[:], in_=lnwb[1:2, :])

            ones1 = sc_pool.tile([1, BATCH], F16, tag="ones1", name="ones1")
            nc.vector.memset(ones1[:], 1.0)
            cb = sc_pool.tile([128, 1], F32, tag="cb", name="cb")
            nc.vector.memset(cb[:, 0:1], 1e-5)

            # conv + out weights on the gpsimd queue
            wc_sb = []
            for m, (c0, cw) in enumerate(CH):
                t = wgt_pool.tile([cw, D_CONV * GC], F16, tag=f"wc{m}",
                                  name=f"wc{m}")
                nc.gpsimd.dma_start(out=t[:], in_=wc[c0:c0 + cw, :])
                wc_sb.append(t)


            # merged activation tiles: [128, 3*N] spanning channel-chunks
            xi_sb = [act_pool.tile([cw, TOK], F16, tag=f"xi{m}", name=f"xi{m}")
                     for m, (c0, cw) in enumerate(CH)]
            xc_all = act_pool.tile([128, 3 * TOKC], F16, tag="xc", name="xc")
            sigz_all = act_pool.tile([128, 3 * TOKZ], F16, tag="sigz",
                                     name="sigz")
            usp_all = act_pool.tile([128, 3 * TOKZ], F16, tag="usp", name="usp")
            nc.vector.memset(usp_all[64:128, 2 * TOKZ:3 * TOKZ], 0.0)
            nc.vector.memset(sigz_all[64:128, 2 * TOKZ:3 * TOKZ], 0.0)
            nc.vector.memset(xc_all[64:128, 2 * TOKC:3 * TOKC], 0.0)
            dt_all = act_pool.tile([128, 3 * TOKZ], F32, tag="dt", name="dt")
            EE = ek_pool.tile([128, 7 * 3 * TOKZ], BF16, tag="EE", name="EE")

            def xc3(m, t0, t1):
                return xc_all[0:CH[m][1], m * TOKC + t0:m * TOKC + t1]

            def xcv(k):     # [128, 3, 512] window view
                v = xc_all[:, :].rearrange("p (m t) -> p m t", m=3)
                return v[:, :, k * BATCH:k * BATCH + TOKZ]

            def eev(k):     # [128, 3, 512] view of plane k
                v = EE[:, (k - 1) * 3 * TOKZ:k * 3 * TOKZ]
                return v.rearrange("p (m t) -> p m t", m=3)

            def ee2(k):     # [128, 1536] view of plane k
                return EE[:, (k - 1) * 3 * TOKZ:k * 3 * TOKZ]

            # ---- A-pass over hi tokens (448..960 + 960..1152 share one
            # ---- ldweights per (k, m)), then conv(448..960), xp, AR --------
            pxi = [ps_pool.tile([cw, 512], F32, tag=f"pxi{m}",
                                name=f"pxi{m}_hi")
                   for m, (c0, cw) in enumerate(CH)]
            pxj = [ps_pool.tile([cw, 192], F32, tag="pc", bufs=3,
                                name=f"pxj{m}_hi")
                   for m, (c0, cw) in enumerate(CH)]
            for k in range(NK):
                for m, (c0, cw) in enumerate(CH):
                    nc.tensor.matmul(pxi[m][:], w_xi(k, c0, cw),
                                     xt_hi(k, 448, 960),
                                     start=(k == 0), stop=(k == NK - 1))
                    nc.tensor.matmul(pxj[m][:], w_xi(k, c0, cw),
                                     xt_hi(k, 960, 1152),
                                     start=(k == 0), stop=(k == NK - 1),
                                     skip_group_check=True)
            for m, (c0, cw) in enumerate(CH):
                nc.scalar.activation(xi_sb[m][:, 448:960], pxi[m][:],
                                     AF.Identity, bias=bias_sb[m][:, 0:1])
                nc.scalar.activation(xi_sb[m][:, 960:1152], pxj[m][:],
                                     AF.Identity, bias=bias_sb[m][:, 0:1])

            def a_pass(t0, t1, xt_f):
                pxa = [ps_pool.tile([cw, t1 - t0], F32, tag=f"pxi{m}",
                                    name=f"pxi{m}_{t0}")
                       for m, (c0, cw) in enumerate(CH)]
                for k in range(NK):
                    for m, (c0, cw) in enumerate(CH):
                        nc.tensor.matmul(pxa[m][:], w_xi(k, c0, cw),
                                         xt_f(k, t0, t1),
                                         start=(k == 0), stop=(k == NK - 1))
                for m, (c0, cw) in enumerate(CH):
                    nc.scalar.activation(xi_sb[m][:, t0:t1], pxa[m][:],
                                         AF.Identity, bias=bias_sb[m][:, 0:1])

            def conv(t0, tw):           # conv outputs for tokens [t0, t0+tw)
                for m, (c0, cw) in enumerate(CH):
                    pc = ps_pool.tile([cw, tw], F32, tag="pc", bufs=3,
                                      name=f"pconv{t0}_{m}")
                    for kc, (k0, kw) in enumerate(CH):
                        for j in range(D_CONV):
                            nc.tensor.matmul(
                                pc[:],
                                wc_sb[kc][:, j * GC + c0:j * GC + c0 + cw],
                                xi_sb[kc][:, t0 + j * BATCH:
                                           t0 + j * BATCH + tw],
                                start=(kc == 0 and j == 0),
                                stop=(kc == 2 and j == D_CONV - 1))
                    nc.scalar.activation(xc3(m, t0, t0 + tw), pc[:], AF.Silu,
                                         bias=bias_sb[m][:, 1:2])

            conv(448, 512)

            pxp = ps_pool.tile([DT_RANK, TOKZ], F32, tag="pc", bufs=3,
                               name="pxp")
            for kc, (k0, kw) in enumerate(CH):
                nc.tensor.matmul(pxp[:], wx_sb[kc][:], xc3(kc, 448, 960),
                                 start=(kc == 0), stop=(kc == 2))
            xp_sb = sc_pool.tile([DT_RANK, TOKZ], F16, tag="xp", name="xp")
            nc.scalar.activation(xp_sb[:], pxp[:], AF.Identity,
                                 bias=bxp_sb[:, 0:1])
            nc.sync.dma_start(out=xp_part[:], in_=xp_sb[:])
            nc.gpsimd.collective_compute(
                "AllReduce", ALU.add, replica_groups=[list(range(8))],
                ins=[xp_part.opt()], outs=[xp_red.opt()])
            xps = sc_pool.tile([DT_RANK, TOKZ], F16, tag="xps", name="xps")
            nc.gpsimd.dma_start(out=xps[:], in_=xp_red[:])
            wo_rows = [(0, 128), (128, 128), (256, 95), (351, 1)]
            wo_sb = []
            for r, (r0, rw) in enumerate(wo_rows):
                t = wo_pool.tile([rw, D_MODEL + 1], F16, tag=f"wo{r}",
                                 name=f"wo{r}")
                nc.gpsimd.dma_start(out=t[:], in_=wo[r0:r0 + rw, :])
                wo_sb.append(t)

            # ---- while the AR flies: rest of A, conv(0..448), dt-proj, gate
            a_pass(0, 448, xt_lo)
            # W_gate loads reuse the W_in tile slots: the WAR dependency on
            # the A-pass reads keeps these transfers out of the contended
            # startup window (they land just before the gate needs them).
            for p in range(NK // 4):
                tz = wig_pool.tile([128, 4, GC], F16, tag=f"wxi{p}",
                                   name=f"wz{p}")
                nc.sync.dma_start(
                    out=tz[:, :, :],
                    in_=wig[512 * p:512 * (p + 1), GC:].rearrange(
                        "(four p) t -> p four t", four=4))
                wz_sb.append(tz)
            conv(0, 448)

            for m, (c0, cw) in enumerate(CH):
                pz = ps_pool.tile([cw, TOKZ], F32, tag="pc", bufs=3, name="pz")
                for k in range(NK):
                    nc.tensor.matmul(pz[:], w_z(k, c0, cw),
                                     xt_hi(k, TOK - TOKZ, TOK),
                                     start=(k == 0), stop=(k == NK - 1))
                nc.scalar.activation(sigz_all[0:cw, m * TOKZ:(m + 1) * TOKZ],
                                     pz[:], AF.Sigmoid,
                                     bias=bias_sb[m][:, 2:3])

            for m, (c0, cw) in enumerate(CH):
                pdt = ps_pool.tile([cw, TOKZ], F32, tag="pc", bufs=3,
                                   name=f"pdt{m}")
                nc.tensor.matmul(pdt[:], wdt_sb[:, c0:c0 + cw], xps[:],
                                 start=True, stop=True)
                nc.scalar.activation(usp_all[0:cw, m * TOKZ:(m + 1) * TOKZ],
                                     pdt[:], AF.Exp)
            # E_k = (1+e^u)^k via one Identity + squares (all same act table)
            nc.scalar.activation(ee2(1), usp_all[:], AF.Identity, bias=1.0)
            nc.scalar.activation(ee2(2), ee2(1), AF.Square)
            nc.scalar.activation(ee2(4), ee2(2), AF.Square)

            # ---- windowed softmax on merged tiles --------------------------
            # S = 1 + sum_k E_k ; num = sum_k E_k * xc<<k ; E_k from Act.
            W3 = 3 * TOKZ
            S = act_pool.tile([128, W3], BF16, tag="S", name="S")
            num = act_pool.tile([128, W3], BF16, tag="num", name="num")
            r3 = lambda ap: ap.rearrange("p (m t) -> p m t", m=3)
            # odd powers + S = 1 + sum E_k, interleaved so S closes early
            nc.vector.tensor_scalar_add(S[:], ee2(1), 1.0)
            nc.vector.tensor_mul(ee2(3), ee2(1), ee2(2))
            nc.vector.tensor_add(S[:], S[:], ee2(2))
            nc.vector.tensor_add(S[:], S[:], ee2(3))
            nc.vector.tensor_mul(ee2(5), ee2(1), ee2(4))
            nc.scalar.activation(ee2(6), ee2(3), AF.Square)
            nc.vector.tensor_add(S[:], S[:], ee2(4))
            nc.vector.tensor_add(S[:], S[:], ee2(5))
            nc.vector.tensor_mul(ee2(7), ee2(3), ee2(4))
            nc.vector.tensor_add(S[:], S[:], ee2(6))
            nc.vector.tensor_add(S[:], S[:], ee2(7))
            # pairwise product tree for num
            tvs = {}
            for k in range(1, 8):
                tv = tmp_pool.tile([128, W3], BF16, tag="tmp", bufs=8,
                                   name=f"t{k}")
                nc.vector.tensor_mul(r3(tv[:]), eev(k), xcv(k))
                tvs[k] = tv
            a10 = tvs[1]
            nc.vector.tensor_add(r3(a10[:]), r3(tvs[1][:]), xcv(0))
            a32 = tvs[3]
            nc.vector.tensor_add(a32[:], tvs[3][:], tvs[2][:])
            a54 = tvs[5]
            nc.vector.tensor_add(a54[:], tvs[5][:], tvs[4][:])
            a76 = tvs[7]
            nc.vector.tensor_add(a76[:], tvs[7][:], tvs[6][:])
            nc.vector.tensor_add(a10[:], a10[:], a32[:])
            nc.vector.tensor_add(a54[:], a54[:], a76[:])
            nc.vector.tensor_add(num[:], a10[:], a54[:])

            sf = tmp_pool.tile([128, W3], F32, tag="sf", bufs=1, name="sf")
            nc.scalar.copy(sf[:], S[:])
            sinv = tmp_pool.tile([128, W3], F32, tag="sinv", bufs=1,
                                 name="sinv")
            nc.vector.reciprocal_approx_fast(out=sinv[:], in_=sf[:])
            loc = tmp_pool.tile([128, W3], F16, tag="loc", bufs=1, name="loc")
            nc.vector.tensor_mul(loc[:], num[:], sinv[:])
            ys = tmp_pool.tile([128, W3], F16, tag="ys", bufs=1, name="ys")
            for m, (c0, cw) in enumerate(CH):
                nc.vector.scalar_tensor_tensor(
                    ys[0:cw, m * TOKZ:(m + 1) * TOKZ],
                    xc3(m, 448, 960), bias_sb[m][:, 3:4],
                    loc[0:cw, m * TOKZ:(m + 1) * TOKZ],
                    op0=ALU.mult, op1=ALU.add)
            nc.vector.tensor_mul(ys[:], ys[:], sigz_all[:])
            ysv = ys[:, :].rearrange("p (m two t) -> p m two t", m=3, two=2)
            tr1 = tmp_pool.tile([128, 3 * 256], F16, tag="tr1", bufs=1, name="tr1")
            nc.vector.tensor_add(
                tr1[:, :].rearrange("p (m t) -> p m t", m=3),
                ysv[:, :, 0], ysv[:, :, 1])
            t1v = tr1[:, :].rearrange("p (m two t) -> p m two t", m=3, two=2)
            tr2 = tmp_pool.tile([128, 3 * 128], F16, tag="tr2", bufs=1, name="tr2")
            nc.vector.tensor_add(
                tr2[:, :].rearrange("p (m t) -> p m t", m=3),
                t1v[:, :, 0], t1v[:, :, 1])
            t2v = tr2[:, :].rearrange("p (m two t) -> p m two t", m=3, two=2)
            cext_all = sc_pool.tile([128, 3 * BATCH], F16, tag="cext",
                                    name="cext")
            nc.vector.tensor_add(
                cext_all[:, :].rearrange("p (m t) -> p m t", m=3),
                t2v[:, :, 0], t2v[:, :, 1])



            # ---- out partial = cext @ woT (+b_out row), AllReduce ----------
            po = [ps_pool.tile([BATCH, 512], F32,
                               tag=(f"pxi{n}" if n < 3 else "po3"),
                               name=f"po{n}")
                  for n in range(4)]
            pomu = ps_pool.tile([BATCH, 1], F32, tag="pc", bufs=3,
                                name="pomu")
            for kc in range(4):
                lhs = (cext_all[0:CH[kc][1], kc * BATCH:(kc + 1) * BATCH]
                       if kc < 3 else ones1[:])
                for n in range(4):
                    nc.tensor.matmul(po[n][:], lhs,
                                     wo_sb[kc][:, n * 512:(n + 1) * 512],
                                     start=(kc == 0), stop=(kc == 3))
                nc.tensor.matmul(pomu[:], lhs,
                                 wo_sb[kc][:, D_MODEL:D_MODEL + 1],
                                 start=(kc == 0), stop=(kc == 3),
                                 skip_group_check=True)
            outp = sc_pool.tile([BATCH, D_MODEL + 1], F16, tag="outp",
                                name="outp")
            osb = sc_pool.tile([BATCH, D_MODEL + 1], F16, tag="osb",
                               name="osb")
            for n in range(2):
                nc.scalar.activation(outp[:, n * 512:(n + 1) * 512],
                                     po[n][:], AF.Copy)
            for n in range(2, 4):
                nc.vector.tensor_copy(outp[:, n * 512:(n + 1) * 512],
                                      po[n][:])
            nc.scalar.activation(outp[:, D_MODEL:D_MODEL + 1], pomu[:],
                                 AF.Copy)
            nc.sync.dma_start(out=op_part[:], in_=outp[:])
            nc.gpsimd.collective_compute(
                "AllReduce", ALU.add, replica_groups=[list(range(8))],
                ins=[op_part.opt()], outs=[op_red.opt()])
            nc.gpsimd.dma_start(out=osb[:], in_=op_red[:])

            # ---- layernorm over d_model (free dim) -------------------------
            mus = sc_pool.tile([BATCH, 1], F32, tag="mus", name="mus")
            nc.scalar.mul(mus[:], osb[:, D_MODEL:D_MODEL + 1], 1.0 / D_MODEL)
            nmus = sc_pool.tile([BATCH, 1], F32, tag="nmus", name="nmus")
            nc.scalar.mul(nmus[:], osb[:, D_MODEL:D_MODEL + 1], -1.0 / D_MODEL)
            cen = sc_pool.tile([BATCH, D_MODEL], F16, tag="cen", name="cen")
            nc.vector.tensor_scalar_sub(cen[:], osb[:, 0:D_MODEL], mus[:])
            sq = sc_pool.tile([BATCH, D_MODEL], F16, tag="outp", name="sq")
            vs = sc_pool.tile([BATCH, 1], F32, tag="vs", name="vs")
            nc.scalar.activation(sq[:], osb[:, 0:D_MODEL], AF.Square,
                                 bias=nmus[:, 0:1], accum_out=vs[:])
            std = sc_pool.tile([BATCH, 1], F32, tag="std", name="std")
            nc.scalar.activation(std[:], vs[:], AF.Sqrt,
                                 scale=1.0 / D_MODEL, bias=cb[0:BATCH, 0:1])
            rstd = sc_pool.tile([BATCH, 1], F32, tag="rstd", name="rstd")
            nc.vector.reciprocal(rstd[:], std[:])
            for n in range(4):
                pw = ps_pool.tile([BATCH, 512], F32,
                                  tag=(f"pxi{n}" if n < 3 else "po3"),
                                  name="pw")
                pb = ps_pool.tile([BATCH, 512], F32, tag="pc", bufs=3,
                                  name="pb")
                nc.tensor.matmul(pw[:], ones1[:],
                                 lnw_sb[:, n * 512:(n + 1) * 512],
                                 start=True, stop=True)
                nc.tensor.matmul(pb[:], ones1[:],
                                 lnb_sb[:, n * 512:(n + 1) * 512],
                                 start=True, stop=True)
                fin = sc_pool.tile([BATCH, 512], F32, tag="fin", bufs=4,
                                   name=f"fin{n}")
                eng = nc.vector
                eng.scalar_tensor_tensor(
                    fin[:], cen[:, n * 512:(n + 1) * 512], rstd[:], pw[:],
                    op0=ALU.mult, op1=ALU.mult)
                eng.tensor_add(fin[:], fin[:], pb[:])
                nc.sync.dma_start(out=out[:, n * 512:(n + 1) * 512],
                                  in_=fin[:])

    nc.compile()
    return nc


def _host_prep(inputs):
    f = lambda k: np.ascontiguousarray(np.asarray(inputs[k], dtype=np.float32))
    x, W_in, b_in = f("x"), f("W_in"), f("b_in")
    W_gate, b_gate = f("W_gate"), f("b_gate")
    W_conv, b_conv = f("W_conv"), f("b_conv")
    W_xproj, b_xproj = f("W_xproj"), f("b_xproj")
    W_dt, Dparam = f("W_dt"), f("Dparam")
    W_out, b_out = f("W_out"), f("b_out")
    ln_w, ln_b = f("ln_w"), f("ln_b")

    xT = np.ascontiguousarray(
        x[SEQ - NPOS:].reshape(TOK, D_MODEL).T).astype(np.float16)
    lnwb = np.ascontiguousarray(np.stack([ln_w, ln_b])).astype(np.float16)

    in_maps = []
    for g in range(8):
        if g < GROUPS:
            ch = slice(GC * g, GC * (g + 1))
            wigm = np.concatenate([W_in[ch].T, W_gate[ch].T], axis=1)
            wcm = np.ascontiguousarray(
                W_conv[ch].transpose(1, 2, 0).reshape(GC, D_CONV * GC))
            wom = np.zeros((GC + 1, D_MODEL + 1), np.float32)
            wom[:GC, :D_MODEL] = W_out[:, ch].T / float(WIN)
            if g == 0:
                wom[GC, :D_MODEL] = b_out
            wom[:, D_MODEL] = wom[:, :D_MODEL].sum(axis=1)
            wxm = np.ascontiguousarray(W_xproj[:DT_RANK, ch].T)
            wdtm = np.ascontiguousarray(W_dt[ch].T)
            biasm = np.ascontiguousarray(
                np.stack([b_in[ch], b_conv[ch], b_gate[ch], Dparam[ch]], 1))
            bxpm = (b_xproj[:DT_RANK] if g == 0
                    else np.zeros(DT_RANK, np.float32)).reshape(DT_RANK, 1)
            bxpm = np.ascontiguousarray(bxpm)
        else:
            wigm = np.zeros((D_MODEL, 2 * GC), np.float32)
            wcm = np.zeros((GC, D_CONV * GC), np.float32)
            wom = np.zeros((GC + 1, D_MODEL + 1), np.float32)
            wxm = np.zeros((GC, DT_RANK), np.float32)
            wdtm = np.zeros((DT_RANK, GC), np.float32)
            biasm = np.zeros((GC, 4), np.float32)
            bxpm = np.zeros((DT_RANK, 1), np.float32)
        in_maps.append({
            "xT": xT,
            "wig": np.ascontiguousarray(wigm).astype(np.float16),
            "wc": wcm.astype(np.float16),
            "wo": wom.astype(np.float16),
            "wx": wxm.astype(np.float16),
            "wdt": wdtm.astype(np.float16),
            "biasv": biasm, "bxp": bxpm, "lnwb": lnwb,
        })
    return in_maps


def kernel(**inputs):
    if "nc" not in _cache:
        _cache["nc"] = _build()
    in_maps = _host_prep(inputs)
    res = run_bass_kernel_spmd(_cache["nc"], in_maps, list(range(8)))
    return res.results[0]["out"]
